# revision 79
# baseline (speedup 1.0000x reference)
"""Distributed Trainium2 kernel for the AttentionBlock problem.

Sharding (v2): tensor-parallel over heads for QKV+attention (each of the 8
cores owns 2 heads for both batches), sequence-parallel for the V projection
and the output projection (each core owns one 512-row block of the flattened
(B*S) dimension).  Two small (1 MB) AllToAll collectives glue the layouts
together:

  1. V is projected seq-parallel (wide, efficient matmuls), then AllToAll'd
     so every core holds V for its own 2 heads over all 4096 rows.  This
     collective overlaps the Q/K projection + RoPE.
  2. After attention, normalized head outputs are AllToAll'd so every core
     holds all 1024 head-dims for its own 512 rows, then applies the full
     output projection locally.  Outputs concatenate on the host.

Device notes:
- All matmul inputs are bf16, PSUM accumulates f32.
- Attention scores are computed transposed (k on partitions, q free) so the
  softmax exp feeds the PV matmul directly; the softmax denominator comes
  from a leading all-ones column prepended to each head's V block.
- Score matmuls for the two batches of a head are row-tiled into the PE
  array concurrently (each uses 64 of the 128 contraction rows).
- The exp runs on the scalar engine at [128,1024] per score block; with
  2-deep score PSUM and 4-deep output PSUM this fits exactly in 8 banks.
"""

import numpy as np
import ml_dtypes

BF16 = ml_dtypes.bfloat16
H, HD, D, B, S = 16, 64, 1024, 2, 2048
LS = 512            # seq rows per core for V / out projection
NC_ = 8
GS = B * S          # 4096 flattened rows
ROPE_THETA = 10000.0

_COMPILED = None


def _build(stage=3):
    import concourse.bass as bass
    import concourse.mybir as mybir
    import concourse.tile as tile
    from concourse import bacc

    fp32 = mybir.dt.float32
    bf16 = mybir.dt.bfloat16

    nc = bacc.Bacc(
        "TRN2", target_bir_lowering=False, debug=False, num_devices=NC_
    )

    xT = nc.dram_tensor("xT", [D, GS], bf16, kind="ExternalInput")
    wqkv = nc.dram_tensor("wqkv", [D, 384], bf16, kind="ExternalInput")
    woutl = nc.dram_tensor("woutl", [128, D], bf16, kind="ExternalInput")
    cosr = nc.dram_tensor("cosr", [128, GS], bf16, kind="ExternalInput")
    sinr = nc.dram_tensor("sinr", [128, GS], bf16, kind="ExternalInput")
    # per-core partial of the output projection, transposed: rows = out
    # dims, cols = global (b*S + s) rows; host sums the 8 partials.
    outT = nc.dram_tensor("outT", [D, GS], bf16, kind="ExternalOutput")

    Exp = mybir.ActivationFunctionType.Exp
    Copy = mybir.ActivationFunctionType.Copy
    Recip = mybir.ActivationFunctionType.Reciprocal

    with tile.TileContext(nc) as tc:
        dma = nc.default_dma_engine
        _keep = []

        def _single(*args, **kwargs):
            t, f = tc.tile(*args, **kwargs)
            _keep.append(f)
            return t

        # ---- persistent SBUF tensors ----
        wqkv_sb = _single([128, 8, 384], bf16, name="wqkv_sb")
        ident = _single([128, 128], bf16, name="ident")
        cos_sb = _single([128, GS], bf16, name="cos_sb")
        sin_sb = _single([128, GS], bf16, name="sin_sb")
        wout_sb = _single([128, D], bf16, name="wout_sb")
        o1_all = _single([128, GS], bf16, name="o1_all")
        o2_all = _single([128, GS], bf16, name="o2_all")
        # per-batch head-pair tiles: rows 0:64 = head h0, 64:128 = head h1
        qp = [_single([128, S], bf16, name=f"qp{b}") for b in range(2)]
        khp = [_single([128, S], bf16, name=f"khp{b}") for b in range(2)]
        vo_all = _single([128, 32, 130], bf16, name="vo_all")
        # normalized attention outputs: [my 128 head dims, slot = 4b+j, q]
        o_all = _single([128, 8, LS], bf16, name="o_all")

        # ---- input DMAs, in pipeline order ----
        # wqkv and the first xT quarter are split per 128-row d-chunk so the
        # first projection's 8-deep accumulation chain starts after ~160KB
        # instead of waiting out the full 2.75MB (the first matmul was
        # data-gated at ~17.5us in every prior variant).
        xq_pool_cm = tc.tile_pool(name="xq_pool", bufs=2)
        xq_pool = xq_pool_cm.__enter__()
        xq_tiles = []
        xq0 = xq_pool.tile([128, 8, 1024], bf16, tag="xq", name="xq0")
        xq_tiles.append(xq0)
        for d8 in range(8):
            dma.dma_start(out=wqkv_sb[:, d8, :],
                          in_=wqkv[128 * d8:128 * (d8 + 1), :])
            dma.dma_start(out=xq0[:, d8, :],
                          in_=xT[128 * d8:128 * (d8 + 1), 0:1024])
        xq1 = xq_pool.tile([128, 8, 1024], bf16, tag="xq", name="xq1")
        dma.dma_start(
            out=xq1[:],
            in_=xT[:, 1024:2048].rearrange("(c p) s -> p c s", p=128),
        )
        xq_tiles.append(xq1)
        dma.dma_start(out=cos_sb[:, 0:S], in_=cosr[:, 0:S])
        dma.dma_start(out=sin_sb[:, 0:S], in_=sinr[:, 0:S])
        dma.dma_start(out=cos_sb[:, S:GS], in_=cosr[:, S:GS])
        dma.dma_start(out=sin_sb[:, S:GS], in_=sinr[:, S:GS])
        dma.dma_start(out=wout_sb[:], in_=woutl[:])

        from concourse import masks as _masks
        _masks.make_identity(nc, ident[:])

        # vo_all[p, kbg, [v_h0(64) | 1 | v_h1(64) | 1]]
        nc.vector.memset(vo_all[:, :, 64:65], 1.0)
        nc.vector.memset(vo_all[:, :, 129:130], 1.0)

        # ========== phase 1: QKV projection + rope + V transpose ==========
        # Per 512-seq chunk: ps1 = qk_x1 channels, ps2 = qk_x2 channels
        # (both roped), ps3 = v^T channels (PE-transposed into vo_all).
        with (
            tc.tile_pool(name="ps_kq", bufs=6, space="PSUM") as ps_kq,
            tc.tile_pool(name="ps_tr", bufs=2, space="PSUM") as ps_tr,
            tc.tile_pool(name="rope_t", bufs=4) as rope_t,
            tc.tile_pool(name="vtp", bufs=2) as vtp,
        ):
            pending_tr = None
            for q4 in range(4):
                if q4 < 2:
                    xq = xq_tiles[q4]
                else:
                    xq = xq_pool.tile([128, 8, 1024], bf16, tag="xq")
                    dma.dma_start(
                        out=xq[:],
                        in_=xT[:, 1024 * q4:1024 * (q4 + 1)].rearrange(
                            "(c p) s -> p c s", p=128),
                    )
                for s2 in range(2):
                    sl = slice(1024 * q4 + 512 * s2, 1024 * q4 + 512 * (s2 + 1))
                    xsl = slice(512 * s2, 512 * (s2 + 1))
                    ps1 = ps_kq.tile([128, 512], fp32, tag="pskq")
                    for d8 in range(8):
                        nc.tensor.matmul(
                            ps1[:], wqkv_sb[:, d8, 0:128], xq[:, d8, xsl],
                            start=(d8 == 0), stop=(d8 == 7),
                        )
                    ps2 = ps_kq.tile([128, 512], fp32, tag="pskq")
                    for d8 in range(8):
                        nc.tensor.matmul(
                            ps2[:], wqkv_sb[:, d8, 128:256], xq[:, d8, xsl],
                            start=(d8 == 0), stop=(d8 == 7),
                        )
                    ps3 = ps_kq.tile([128, 512], fp32, tag="pskq")
                    for d8 in range(8):
                        nc.tensor.matmul(
                            ps3[:], wqkv_sb[:, d8, 256:384], xq[:, d8, xsl],
                            start=(d8 == 0), stop=(d8 == 7),
                        )
                    cs, sn = cos_sb[:, sl], sin_sb[:, sl]
                    t1 = rope_t.tile([128, 512], bf16, tag="rt")
                    t2 = rope_t.tile([128, 512], bf16, tag="rt")
                    nc.vector.tensor_mul(t1[:], ps1[:], cs)
                    nc.vector.tensor_mul(t2[:], ps2[:], sn)
                    nc.vector.tensor_sub(o1_all[:, sl], t1[:], t2[:])
                    nc.vector.tensor_mul(t1[:], ps1[:], sn)
                    nc.vector.tensor_mul(t2[:], ps2[:], cs)
                    nc.vector.tensor_add(o2_all[:, sl], t1[:], t2[:])
                    # v^T -> bf16 staging; the PE transposes are deferred
                    # by one chunk so they never stall on this copy.
                    vt = vtp.tile([128, 512], bf16, tag="vt")
                    nc.scalar.activation(vt[:], ps3[:], Copy)
                    if pending_tr is not None:
                        pq4, ps2_, pvt = pending_tr
                        for t4 in range(4):
                            kbg = 8 * pq4 + 4 * ps2_ + t4
                            ptr = ps_tr.tile([128, 128], bf16, tag="ptr")
                            nc.tensor.transpose(
                                ptr[:], pvt[:, 128 * t4:128 * (t4 + 1)],
                                ident[:])
                            nc.vector.tensor_copy(
                                vo_all[:, kbg, 0:130].rearrange(
                                    "p (two c) -> p two c",
                                    two=2)[:, :, 0:64],
                                ptr[:].rearrange("p (two c) -> p two c",
                                                 two=2))
                    pending_tr = (q4, s2, vt)
                if q4 % 2 == 1:
                    # batch bb fully roped: assemble its head-pair tiles
                    # qp/khp rows: 0:32 h0_x1', 32:64 h0_x2', 64:96 h1_x1',
                    # 96:128 h1_x2'
                    bb = q4 // 2
                    bsl = slice(S * bb, S * (bb + 1))
                    for hl in range(2):
                        dma.dma_start(
                            out=qp[bb][64 * hl:64 * hl + 32, :],
                            in_=o1_all[32 * hl:32 * (hl + 1), bsl])
                        dma.dma_start(
                            out=qp[bb][64 * hl + 32:64 * hl + 64, :],
                            in_=o2_all[32 * hl:32 * (hl + 1), bsl])
                        dma.dma_start(
                            out=khp[bb][64 * hl:64 * hl + 32, :],
                            in_=o1_all[64 + 32 * hl:64 + 32 * (hl + 1), bsl])
                        dma.dma_start(
                            out=khp[bb][64 * hl + 32:64 * hl + 64, :],
                            in_=o2_all[64 + 32 * hl:64 + 32 * (hl + 1), bsl])

            pq4, ps2_, pvt = pending_tr
            for t4 in range(4):
                kbg = 8 * pq4 + 4 * ps2_ + t4
                ptr = ps_tr.tile([128, 128], bf16, tag="ptr",
                                 name=f"ptrf{t4}")
                nc.tensor.transpose(
                    ptr[:], pvt[:, 128 * t4:128 * (t4 + 1)], ident[:])
                nc.vector.tensor_copy(
                    vo_all[:, kbg, 0:130].rearrange(
                        "p (two c) -> p two c", two=2)[:, :, 0:64],
                    ptr[:].rearrange("p (two c) -> p two c", two=2))

        xq_pool_cm.__exit__(None, None, None)

        if stage == 1:
            with tc.tile_pool(name="dbg", bufs=2) as dbg:
                o = dbg.tile([128, LS], fp32, name="dbgo")
                nc.vector.tensor_copy(o[:, 0:64], vo_all[:, 0, 0:64])
                nc.vector.tensor_copy(o[:, 64:128], vo_all[:, 17, 65:129])
                nc.vector.memset(o[:, 128:LS], 0.0)
                dma.dma_start(out=outT[0:128, :], in_=o[:])
                o2 = dbg.tile([128, LS], fp32, name="dbgo2")
                nc.vector.tensor_copy(o2[:], qp[0][:, 0:LS])
                dma.dma_start(out=outT[128:256, :], in_=o2[:])
                o3 = dbg.tile([128, LS], fp32, name="dbgo3")
                nc.vector.tensor_copy(o3[:], khp[0][:, 0:LS])
                dma.dma_start(out=outT[256:384, :], in_=o3[:])
            for f in reversed(_keep):
                f()
            nc.compile()
            return nc

        # ================= phase 3: attention ==============================
        with (
            tc.tile_pool(name="ps_sc", bufs=2, space="PSUM") as ps_sc,
            tc.tile_pool(name="ps_out", bufs=3, space="PSUM") as ps_out,
            tc.tile_pool(name="ps_op", bufs=1, space="PSUM") as ps_op,
            tc.tile_pool(name="p_pool", bufs=32) as p_pool,
            tc.tile_pool(name="fin", bufs=4) as fin,
            tc.tile_pool(name="ocp", bufs=4) as ocp,
        ):
            def oproj_step(slot, oc):
                # one 128-col chunk of the partial output projection; its
                # matmuls are spread between attention score/PV matmuls so
                # they never stall the exp stream, and its copies run on DVE.
                ps = ps_op.tile([128, 512], fp32, tag="psop")
                nc.tensor.matmul(
                    ps[:], wout_sb[:, 128 * oc:128 * (oc + 1)],
                    o_all[:, slot, :],
                    start=True, stop=True,
                )
                ot = ocp.tile([128, 512], bf16, tag="ocp")
                nc.vector.tensor_copy(ot[:], ps[:])
                dma.dma_start(
                    out=outT[128 * oc:128 * (oc + 1),
                             512 * slot:512 * (slot + 1)],
                    in_=ot[:])

            def emit_job(bb, j, oslot):
                qsl = slice(512 * j, 512 * (j + 1))
                outp = [ps_out.tile([128, 512], fp32, tag="pso",
                                    name=f"outp{bb}_{j}_{u}")
                        for u in range(2)]
                for kb in range(16):
                    sc_ps = ps_sc.tile([128, 1024], fp32, tag="sc")
                    ksl = slice(128 * kb, 128 * (kb + 1))
                    for u in range(2):
                        nc.tensor.matmul(
                            sc_ps[:, 512 * u:512 * (u + 1)],
                            khp[bb][64 * u:64 * (u + 1), ksl],
                            qp[bb][64 * u:64 * (u + 1), qsl],
                            start=True, stop=True,
                        )
                    p_sb = p_pool.tile([128, 1024], bf16, tag="p")
                    nc.scalar.activation(p_sb[:], sc_ps[:], Exp, scale=0.125)
                    for u in range(2):
                        nc.tensor.matmul(
                            outp[u][0:65, :],
                            vo_all[:, 16 * bb + kb, 65 * u:65 * (u + 1)],
                            p_sb[:, 512 * u:512 * (u + 1)],
                            start=(kb == 0), stop=(kb == 15),
                            skip_group_check=True,
                        )
                    if oslot is not None and kb % 2 == 1:
                        oproj_step(oslot, kb // 2)
                slot = 4 * bb + j
                for u in range(2):
                    dsb = fin.tile([1, 512], fp32, tag="dsb")
                    nc.vector.tensor_copy(dsb[:], outp[u][64:65, :])
                    recip = fin.tile([1, 512], fp32, tag="recip")
                    nc.vector.reciprocal_approx_fast(recip[:], dsb[:])
                    bcast = fin.tile([64, 512], fp32, tag="bcast")
                    nc.gpsimd.partition_broadcast(bcast[:], recip[:])
                    nc.vector.tensor_mul(
                        o_all[64 * u:64 * (u + 1), slot, :],
                        outp[u][0:64, :], bcast[:])

            sched = [(0, 0, None), (0, 1, None), (0, 2, 0), (0, 3, 1),
                     (1, 0, 2), (1, 1, 3), (1, 2, 4), (1, 3, 5)]
            for bb, j, opr in sched:
                emit_job(bb, j, opr)

            # last two slots drain through the (now free) score pool,
            # two output chunks per 2-bank tile.
            for slot in (6, 7):
                for oc2 in range(4):
                    ps = ps_sc.tile([128, 1024], fp32, tag="sc")
                    for half in range(2):
                        oc = 2 * oc2 + half
                        nc.tensor.matmul(
                            ps[:, 512 * half:512 * (half + 1)],
                            wout_sb[:, 128 * oc:128 * (oc + 1)],
                            o_all[:, slot, :],
                            start=True, stop=True,
                        )
                    ot = ocp.tile([128, 1024], bf16, tag="ocp2")
                    nc.vector.tensor_copy(ot[:], ps[:])
                    dma.dma_start(
                        out=outT[256 * oc2:256 * (oc2 + 1),
                                 512 * slot:512 * (slot + 1)].rearrange(
                                     "(two p) q -> p two q", two=2),
                        in_=ot[:])

        for f in reversed(_keep):
            f()

    nc.compile()
    return nc


def _host_prep(inputs, positions, w_in, w_out):
    inputs = np.asarray(inputs, np.float32)
    positions = np.asarray(positions)
    w_in = np.asarray(w_in, np.float32)
    w_out = np.asarray(w_out, np.float32)

    x_all = np.concatenate([inputs[0], inputs[1]], axis=0)          # (4096, D)
    xT_full = np.ascontiguousarray(x_all.T).astype(BF16)            # (D, 4096)

    ar32, ar64 = np.arange(32), np.arange(64)

    inv_freq = 1.0 / (ROPE_THETA ** (np.arange(32, dtype=np.float32) / 32))
    pos_all = np.concatenate([positions[0], positions[1]]).astype(np.float32)
    ang = pos_all[None, :] * inv_freq[:, None]                      # (32, 4096)
    cosr = np.ascontiguousarray(np.tile(np.cos(ang), (4, 1))).astype(BF16)
    sinr = np.ascontiguousarray(np.tile(np.sin(ang), (4, 1))).astype(BF16)

    in_maps = []
    for c in range(NC_):
        H0, H1 = 2 * c, 2 * c + 1
        cols = np.concatenate([
            192 * H0 + ar32, 192 * H1 + ar32,            # q_x1 h0, h1
            192 * H0 + 64 + ar32, 192 * H1 + 64 + ar32,  # k_x1 h0, h1
            192 * H0 + 32 + ar32, 192 * H1 + 32 + ar32,  # q_x2 h0, h1
            192 * H0 + 96 + ar32, 192 * H1 + 96 + ar32,  # k_x2 h0, h1
            192 * H0 + 128 + ar64, 192 * H1 + 128 + ar64,  # v h0, h1
        ])
        wqkv = np.ascontiguousarray(w_in[:, cols]).astype(BF16)
        # rows of w_out for my two heads' output dims
        woutl = np.ascontiguousarray(
            w_out[128 * c:128 * (c + 1), :]).astype(BF16)
        in_maps.append({
            "xT": xT_full, "wqkv": wqkv,
            "woutl": woutl, "cosr": cosr, "sinr": sinr,
        })
    return in_maps


def kernel(inputs, positions, w_in, w_out, _trace=False):
    global _COMPILED
    from concourse.bass_utils import run_bass_kernel_spmd

    if _COMPILED is None:
        _COMPILED = _build()
    nc = _COMPILED

    in_maps = _host_prep(inputs, positions, w_in, w_out)
    res = run_bass_kernel_spmd(
        nc, in_maps, core_ids=list(range(NC_)), trace=_trace
    )
    kernel.last_results = res

    acc = np.zeros((D, GS), np.float32)
    for c in range(NC_):
        acc += np.asarray(res.results[c]["outT"], dtype=np.float32)
    return np.ascontiguousarray(acc.T).reshape(B, S, D)
